# revision 1
# baseline (speedup 1.0000x reference)
"""NeuroSAT GNN message passing on 8 Trainium2 NeuronCores.

Strategy (graph-data-parallel, 2 graphs per core, zero collectives):
  * The 3-layer MLPs in the reference have no nonlinearity -> each collapses
    to one 64x64 linear, folded on the host into the LSTM input projections.
  * Per-graph scatter-add aggregation over the bipartite clause<->literal
    edges is a dense matmul with the per-graph 440x800 incidence matrix
    (built on host from edge_index). Self-loops fold into the recurrent
    weights; literal-degree bias becomes a host-precomputed [128,800] matrix
    added with one DVE op per gate.
  * Clause LSTM state is only read at clause rows and literal LSTM state at
    literal rows, so each LSTM runs on only its 440/800 rows per graph.
  * flip_perm is a per-graph half-swap of literal columns -> realized by
    reading swapped column ranges.

Layout: the core's 2 graphs share the partition axis: graph0 on partitions
0:64, graph1 on 64:128 of every [128, N] tile (feature-major per half).
Gate matmuls are M=64 per gate, column-tiled (tile_position) so both
graphs' matmuls co-run on the two PE-array column halves. All matmuls are
fp32 (fp32r is tf32 and the recurrence is chaotic: ~5e3x amplification).
"""

import numpy as np

H = 64
ITERS = 24
B, NV, NC, K = 16, 400, 440, 12
NL = 2 * NV                  # literals/graph = 800
NPG = NL + NC                # nodes/graph = 1240
N = B * NPG                  # 19840
NCORES = 8
GPC = B // NCORES            # graphs per core = 2
CHK = 400                    # literal column chunk (aligned to NV flip halves)

_PROGRAM_CACHE = {}


def _build_program():
    from contextlib import ExitStack

    import concourse.bacc as bacc
    import concourse.mybir as mybir
    from concourse.masks import make_identity
    from concourse.tile import TileContext, add_dep_helper

    F32 = mybir.dt.float32
    SIG = mybir.ActivationFunctionType.Sigmoid
    MULT = mybir.AluOpType.mult
    SUB = mybir.AluOpType.subtract

    nc = bacc.Bacc(
        "TRN2", target_bir_lowering=False, debug=False, num_devices=NCORES
    )

    # ---- DRAM I/O (per-core shards; weights replicated) ----
    d_xt_lit = nc.dram_tensor("xt_lit", [3, GPC * NL], F32, kind="ExternalInput")
    d_xt_cl = nc.dram_tensor("xt_cl", [3, GPC * NC], F32, kind="ExternalInput")
    d_at = nc.dram_tensor("at_rm", [GPC, 7, 128, NC], F32, kind="ExternalInput")
    d_a = nc.dram_tensor("a_rm", [GPC, 4, 128, NL], F32, kind="ExternalInput")
    d_wca = nc.dram_tensor("wc_a", [128, 256], F32, kind="ExternalInput")
    d_wcb = nc.dram_tensor("wc_b", [128, 256], F32, kind="ExternalInput")
    d_wc1 = nc.dram_tensor("wc_1", [128, 256], F32, kind="ExternalInput")
    d_wla = nc.dram_tensor("wl_a", [128, 256], F32, kind="ExternalInput")
    d_wlb = nc.dram_tensor("wl_b", [128, 256], F32, kind="ExternalInput")
    d_wlh = nc.dram_tensor("w_lh_dup", [128, 256], F32, kind="ExternalInput")
    d_wcl2 = nc.dram_tensor("w_cl2_dup", [128, 256], F32, kind="ExternalInput")
    d_wv = nc.dram_tensor("wv_dup", [128, 1], F32, kind="ExternalInput")
    d_liw = nc.dram_tensor("li_w3", [3, H], F32, kind="ExternalInput")
    d_ciw = nc.dram_tensor("ci_w3", [3, H], F32, kind="ExternalInput")
    d_bias = nc.dram_tensor("bias_dup", [128, 5], F32, kind="ExternalInput")
    d_dqq = nc.dram_tensor("dqq", [128, 4 * NL], F32, kind="ExternalInput")
    d_out = nc.dram_tensor("vote", [1, GPC * NL], F32, kind="ExternalOutput")

    with TileContext(nc) as tc, ExitStack() as ctx:
        const = ctx.enter_context(tc.tile_pool(name="const", bufs=1))
        state = ctx.enter_context(tc.tile_pool(name="state", bufs=2))
        work = ctx.enter_context(tc.tile_pool(name="work", bufs=2))
        ps = ctx.enter_context(tc.tile_pool(name="ps", bufs=1, space="PSUM"))

        # ---- constants to SBUF ----
        ident = const.tile([128, 128], F32, name="ident")
        make_identity(nc, ident)

        # adjacency chunks are host-padded so every chunk is a full 128 rows
        # (last chunk overlaps the previous one with zeroed overlap rows)
        at_t = const.tile([128, GPC * 7 * NC], F32, name="at_t")  # A^T k-chunks
        for g in range(GPC):
            for k in range(7):
                c0 = NC * (7 * g + k)
                nc.sync.dma_start(out=at_t[:, c0:c0 + NC], in_=d_at[g, k])
        a_t = const.tile([128, GPC * 4 * NL], F32, name="a_t")  # A k-chunks
        for g in range(GPC):
            for k in range(4):
                c0 = NL * (4 * g + k)
                nc.sync.dma_start(out=a_t[:, c0:c0 + NL], in_=d_a[g, k])

        def load(dram, shape, nm):
            t = const.tile(shape, F32, name=nm)
            nc.sync.dma_start(out=t[:, :], in_=dram[:, :])
            return t

        xt_lit = load(d_xt_lit, [3, GPC * NL], "xt_lit_sb")
        xt_cl = load(d_xt_cl, [3, GPC * NC], "xt_cl_sb")
        wc_a = load(d_wca, [128, 256], "wc_a_sb")
        wc_b = load(d_wcb, [128, 256], "wc_b_sb")
        wc_1 = load(d_wc1, [128, 256], "wc_1_sb")
        wl_a = load(d_wla, [128, 256], "wl_a_sb")
        wl_b = load(d_wlb, [128, 256], "wl_b_sb")
        w_lh = load(d_wlh, [128, 256], "w_lh_sb")
        w_cl2 = load(d_wcl2, [128, 256], "w_cl2_sb")
        wv = load(d_wv, [128, 1], "wv_sb")
        li_w = load(d_liw, [3, H], "li_w_sb")
        ci_w = load(d_ciw, [3, H], "ci_w_sb")
        bias = load(d_bias, [128, 5], "bias_sb")
        dqq = load(d_dqq, [128, 4 * NL], "dqq_sb")

        def MM(*a, **kw):
            kw.setdefault("skip_group_check", True)
            return nc.tensor.matmul(*a, **kw)

        TPOS = ((0, 0), (0, 64))  # col-group per graph-half
        LO, HI = slice(0, 64), slice(64, 128)
        HALF = (LO, HI)

        # ---- initial node states (bias via ones row of xt) ----
        lit_h = state.tile([128, NL], F32, tag="lit_h", name="lit_h0")
        for hf in range(2):
            p = ps.tile([128, CHK], F32, tag="g", bufs=4, name=f"ini_{hf}")
            for g in range(GPC):
                MM(p[HALF[g], :], li_w[0:3, :],
                   xt_lit[0:3, g * NL + hf * CHK:g * NL + (hf + 1) * CHK],
                   start=True, stop=True, tile_position=TPOS[g])
            nc.scalar.copy(lit_h[:, hf * CHK:(hf + 1) * CHK], p[:, :])
        cl_h = state.tile([128, NC], F32, tag="cl_h", name="cl_h0")
        pc = ps.tile([128, NC], F32, tag="g", bufs=4, name="ini_c")
        for g in range(GPC):
            MM(pc[HALF[g], :], ci_w[0:3, :], xt_cl[0:3, g * NC:(g + 1) * NC],
               start=True, stop=True, tile_position=TPOS[g])
        nc.scalar.copy(cl_h[:, :], pc[:, :])

        out_lit = lit_h      # [128, 800]: rows 0:64 g0 features, 64:128 g1
        out_cl = cl_h        # [128, 440]
        lit_c = None
        cl_c = None

        for t in range(1, ITERS):
            first = t == 1

            # -- transpose out_lit -> row-major [lit, feat] chunks per graph --
            rm_l = []
            for g in range(GPC):
                tp = ps.tile([128, 7 * H], F32, tag="ta", bufs=2,
                             name=f"tpl_{t}_{g}")
                for k in range(7):
                    c0 = 128 * k if k < 6 else NL - 128
                    nc.tensor.transpose(
                        tp[:, k * H:(k + 1) * H],
                        out_lit[HALF[g], c0:c0 + 128],
                        ident[HALF[g], HALF[g]],
                    )
                rm = work.tile([128, 7 * H], F32, tag="rml", bufs=3, name=f"rml_{t}_{g}")
                nc.scalar.copy(rm[:, :], tp[:, :])
                rm_l.append(rm)

            # -- clause agg: raw A @ out_lit per graph (g0 -> hi, g1 -> lo) --
            agc = ps.tile([128, NC], F32, tag="ta", bufs=2, name=f"agc_{t}")
            prev = None
            for g in range(GPC):
                half = HI if g == 0 else LO
                for k in range(7):
                    mm = MM(agc[half, :], rm_l[g][:, k * H:(k + 1) * H],
                            at_t[:, NC * (7 * g + k):NC * (7 * g + k + 1)],
                            start=(k == 0), stop=(k == 6),
                            tile_position=TPOS[1 - g])
                    if k == 0 and prev is not None:
                        add_dep_helper(mm.ins, prev.ins, sync=True,
                                       reason="psum half-group order")
                    if k == 6:
                        prev = mm
            # stacks: g0 = (ch | agg), g1 = (agg | ch)
            st0 = work.tile([128, NC], F32, tag="stc0", bufs=3, name=f"stc0_{t}")
            st1 = work.tile([128, NC], F32, tag="stc1", bufs=3, name=f"stc1_{t}")
            nc.gpsimd.tensor_copy(st0[LO, :], out_cl[LO, :])
            nc.scalar.copy(st0[HI, :], agc[HI, :])
            nc.scalar.copy(st1[LO, :], agc[LO, :])
            nc.gpsimd.tensor_copy(st1[HI, :], out_cl[HI, :])

            # -- clause gates: 4 gate groups, col-tiled graph pairs --
            wA, wB = (wc_1, wc_1) if first else (wc_a, wc_b)
            gps_c = []
            for x in range(4):
                gp = ps.tile([128, NC], F32, tag="cg", bufs=2, name=f"cg{x}_{t}")
                xs = slice(x * H, (x + 1) * H)
                MM(gp[LO, :], wA[:, xs], st0[:, :], start=True, stop=True,
                   tile_position=(0, 0))
                MM(gp[HI, :], wB[:, xs], st1[:, :], start=True, stop=True,
                   tile_position=(0, 64))
                gps_c.append(gp)
            ch_new = state.tile([128, NC], F32, tag="cl_h", name=f"ch_{t}")
            cc_new = state.tile([128, NC], F32, tag="cl_c", name=f"cc_{t}")
            # tanh(x) = 2*sigmoid(2x) - 1: keeps ACT on one table (no
            # 1283ns table reload between Sigmoid and Tanh)
            s_i = work.tile([128, NC], F32, tag="si", name=f"csi_{t}")
            nc.scalar.activation(s_i[:, :], gps_c[0][:, :], SIG, bias=bias[:, 0:1])
            s_f = work.tile([128, NC], F32, tag="sf", name=f"csf_{t}")
            nc.scalar.activation(s_f[:, :], gps_c[1][:, :], SIG, bias=bias[:, 1:2])
            s_g = work.tile([128, NC], F32, tag="sg", name=f"csg_{t}")
            nc.scalar.activation(s_g[:, :], gps_c[2][:, :], SIG,
                                 bias=bias[:, 2:3], scale=2.0)
            s_o = work.tile([128, NC], F32, tag="so", name=f"cso_{t}")
            nc.scalar.activation(s_o[:, :], gps_c[3][:, :], SIG, bias=bias[:, 3:4])
            t1 = work.tile([128, NC], F32, tag="t1", name=f"ct1_{t}")
            nc.vector.tensor_mul(t1[:, :], s_i[:, :], s_g[:, :])
            if first:
                nc.vector.scalar_tensor_tensor(
                    cc_new[:, :], t1[:, :], 2.0, s_i[:, :],
                    op0=MULT, op1=SUB)
            else:
                u = work.tile([128, NC], F32, tag="u", name=f"cu_{t}")
                nc.vector.scalar_tensor_tensor(
                    u[:, :], t1[:, :], 2.0, s_i[:, :], op0=MULT, op1=SUB)
                t2 = work.tile([128, NC], F32, tag="t2", name=f"ct2_{t}")
                nc.vector.tensor_mul(t2[:, :], s_f[:, :], cl_c[:, :])
                nc.vector.tensor_add(cc_new[:, :], u[:, :], t2[:, :])
            tnc = work.tile([128, NC], F32, tag="tnc", name=f"ctn_{t}")
            nc.scalar.activation(tnc[:, :], cc_new[:, :], SIG, scale=2.0)
            t3 = work.tile([128, NC], F32, tag="t3", name=f"ct3_{t}")
            nc.vector.tensor_mul(t3[:, :], s_o[:, :], tnc[:, :])
            nc.vector.scalar_tensor_tensor(
                ch_new[:, :], t3[:, :], 2.0, s_o[:, :], op0=MULT, op1=SUB)

            # -- transpose ch_new; literal agg per half-chunk --
            rm_c = []
            for g in range(GPC):
                tp = ps.tile([128, 4 * H], F32, tag="ta", bufs=2,
                             name=f"tpc_{t}_{g}")
                for k in range(4):
                    c0 = 128 * k if k < 3 else NC - 128
                    nc.tensor.transpose(
                        tp[:, k * H:(k + 1) * H],
                        ch_new[HALF[g], c0:c0 + 128],
                        ident[HALF[g], HALF[g]],
                    )
                rm = work.tile([128, 4 * H], F32, tag="rmc", bufs=3, name=f"rmc_{t}_{g}")
                nc.scalar.copy(rm[:, :], tp[:, :])
                rm_c.append(rm)

            lh_new = state.tile([128, NL], F32, tag="lit_h", name=f"lh_{t}")
            lc_new = state.tile([128, NL], F32, tag="lit_c", name=f"lc_{t}")
            wS = w_cl2 if first else w_lh
            for hf in range(2):
                cs = slice(hf * CHK, (hf + 1) * CHK)
                fs = slice((1 - hf) * CHK, (2 - hf) * CHK)
                agl = ps.tile([128, CHK], F32, tag="ta", bufs=2,
                              name=f"agl_{t}_{hf}")
                prev = None
                for g in range(GPC):
                    half = HI if g == 0 else LO
                    for k in range(4):
                        mm = MM(agl[half, :], rm_c[g][:, k * H:(k + 1) * H],
                                a_t[:, NL * (4 * g + k) + hf * CHK:
                                    NL * (4 * g + k) + (hf + 1) * CHK],
                                start=(k == 0), stop=(k == 3),
                                tile_position=TPOS[1 - g])
                        if k == 0 and prev is not None:
                            add_dep_helper(mm.ins, prev.ins, sync=True,
                                           reason="psum half-group order")
                        if k == 3:
                            prev = mm
                # stacks: g0 = (flip | agg), g1 = (agg | flip)
                s0 = work.tile([128, CHK], F32, tag="stl0", bufs=3, name=f"sl0_{t}_{hf}")
                s1 = work.tile([128, CHK], F32, tag="stl1", bufs=3, name=f"sl1_{t}_{hf}")
                nc.gpsimd.tensor_copy(s0[LO, :], out_lit[LO, fs])
                nc.scalar.copy(s0[HI, :], agl[HI, :])
                nc.scalar.copy(s1[LO, :], agl[LO, :])
                nc.gpsimd.tensor_copy(s1[HI, :], out_lit[HI, fs])

                gps = []
                for x in range(4):
                    gp = ps.tile([128, CHK], F32, tag="g", bufs=4,
                                 name=f"lg{x}_{t}_{hf}")
                    xs = slice(x * H, (x + 1) * H)
                    MM(gp[LO, :], wS[0:64, xs], out_lit[LO, cs],
                       start=True, stop=False, tile_position=(0, 0))
                    lo2 = MM(gp[LO, :], wl_a[:, xs], s0[:, :], start=False,
                             stop=True, tile_position=(0, 0))
                    hi1 = MM(gp[HI, :], wS[64:128, xs], out_lit[HI, cs],
                             start=True, stop=False, tile_position=(64, 64))
                    add_dep_helper(hi1.ins, lo2.ins, sync=True,
                                   reason="psum half-group order")
                    MM(gp[HI, :], wl_b[:, xs], s1[:, :], start=False,
                       stop=True, tile_position=(0, 64))
                    # degree-dependent bias + lu biases (host-precomputed);
                    # lands in SBUF so the psum bank frees early
                    stg = work.tile([128, CHK], F32, tag=f"stg{x}",
                                    name=f"stg{x}_{t}_{hf}")
                    nc.vector.tensor_add(stg[:, :], gp[:, :],
                                         dqq[:, x * NL + hf * CHK:
                                             x * NL + (hf + 1) * CHK])
                    gps.append(stg)
                s_i = work.tile([128, CHK], F32, tag="si", name=f"lsi_{t}_{hf}")
                nc.scalar.activation(s_i[:, :], gps[0][:, :], SIG)
                s_f = work.tile([128, CHK], F32, tag="sf", name=f"lsf_{t}_{hf}")
                nc.scalar.activation(s_f[:, :], gps[1][:, :], SIG)
                s_g = work.tile([128, CHK], F32, tag="sg", name=f"lsg_{t}_{hf}")
                nc.scalar.activation(s_g[:, :], gps[2][:, :], SIG, scale=2.0)
                s_o = work.tile([128, CHK], F32, tag="so", name=f"lso_{t}_{hf}")
                nc.scalar.activation(s_o[:, :], gps[3][:, :], SIG)
                t1 = work.tile([128, CHK], F32, tag="t1", name=f"lt1_{t}_{hf}")
                nc.vector.tensor_mul(t1[:, :], s_i[:, :], s_g[:, :])
                if first:
                    nc.vector.scalar_tensor_tensor(
                        lc_new[:, cs], t1[:, :], 2.0, s_i[:, :],
                        op0=MULT, op1=SUB)
                else:
                    u = work.tile([128, CHK], F32, tag="u", name=f"lu_{t}_{hf}")
                    nc.vector.scalar_tensor_tensor(
                        u[:, :], t1[:, :], 2.0, s_i[:, :], op0=MULT, op1=SUB)
                    t2 = work.tile([128, CHK], F32, tag="t2", name=f"lt2_{t}_{hf}")
                    nc.vector.tensor_mul(t2[:, :], s_f[:, :], lit_c[:, cs])
                    nc.vector.tensor_add(lc_new[:, cs], u[:, :], t2[:, :])
                tnc = work.tile([128, CHK], F32, tag="tnc", name=f"ltn_{t}_{hf}")
                nc.scalar.activation(tnc[:, :], lc_new[:, cs], SIG, scale=2.0)
                t3 = work.tile([128, CHK], F32, tag="t3", name=f"lt3_{t}_{hf}")
                nc.vector.tensor_mul(t3[:, :], s_o[:, :], tnc[:, :])
                nc.vector.scalar_tensor_tensor(
                    lh_new[:, cs], t3[:, :], 2.0, s_o[:, :], op0=MULT, op1=SUB)

            out_lit, out_cl = lh_new, ch_new
            lit_c, cl_c = lc_new, cc_new

        # ---- vote head ----
        vote_sb = work.tile([1, GPC * NL], F32, tag="vote", name="vote_sb")
        for g in range(GPC):
            for hf in range(2):
                p = ps.tile([1, CHK], F32, tag="ta", bufs=2,
                            name=f"vps_{g}_{hf}")
                MM(p[:, :], wv[HALF[g], 0:1],
                   out_lit[HALF[g], hf * CHK:(hf + 1) * CHK],
                   start=True, stop=True,
                   tile_position=(64 * g, 0))
                nc.scalar.activation(
                    vote_sb[0:1, g * NL + hf * CHK:g * NL + (hf + 1) * CHK],
                    p[:, :], mybir.ActivationFunctionType.Identity,
                    bias=bias[0:1, 4:5],
                )
        nc.sync.dma_start(out=d_out[:, :], in_=vote_sb[:, :])

    nc.compile()
    return nc


def _fold_and_shard(inputs):
    """Host-side preprocessing: fold weights, build adjacency, shard by graph."""
    f32 = np.float32
    g = {k: np.asarray(v) for k, v in inputs.items()}

    def collapse(w1, b1, w2, b2, w3, b3):
        return w1 @ w2 @ w3, ((b1 @ w2) + b2) @ w3 + b3

    Wl, bl = collapse(g["lm1_w"], g["lm1_b"], g["lm2_w"], g["lm2_b"],
                      g["lm3_w"], g["lm3_b"])
    Wc, bc = collapse(g["cm1_w"], g["cm1_b"], g["cm2_w"], g["cm2_b"],
                      g["cm3_w"], g["cm3_b"])
    Wv, bv = collapse(g["lv1_w"], g["lv1_b"], g["lv2_w"], g["lv2_b"],
                      g["lv3_w"], g["lv3_b"])

    cu_wih, lu_wih = g["cu_wih"], g["lu_wih"]
    w_lc = (Wl @ cu_wih).astype(f32)                 # agg_c -> clause gates
    w_ch = (w_lc + g["cu_whh"]).astype(f32)          # t>=2 merged recurrent
    cbias_c = ((K + 1) * (bl @ cu_wih) + g["cu_bih"] + g["cu_bhh"]).astype(f32)
    wih_a = lu_wih[0:H].astype(f32)                  # flip -> lit gates
    w_cl2 = (Wc @ lu_wih[H:2 * H]).astype(f32)       # agg_l -> lit gates
    w_lh = (w_cl2 + g["lu_whh"]).astype(f32)         # t>=2 merged recurrent
    q_l = (bc @ lu_wih[H:2 * H]).astype(f32)         # [256]
    cbias_l = (g["lu_bih"] + g["lu_bhh"]).astype(f32)

    vs = np.vstack
    wc_a = vs([w_ch, w_lc])
    wc_b = vs([w_lc, w_ch])
    wc_1 = vs([w_lc, w_lc])
    wl_a = vs([wih_a, w_cl2])
    wl_b = vs([w_cl2, wih_a])
    w_lh_dup = vs([w_lh, w_lh])
    w_cl2_dup = vs([w_cl2, w_cl2])
    wv_dup = vs([Wv.astype(f32), Wv.astype(f32)])

    bias_dup = np.zeros((128, 5), f32)
    for x in range(4):
        scl = 2.0 if x == 2 else 1.0   # g-gate runs as sigmoid(2x+2b)
        bias_dup[0:64, x] = scl * cbias_c[x * H:(x + 1) * H]
        bias_dup[64:128, x] = scl * cbias_c[x * H:(x + 1) * H]
    bias_dup[0, 4] = bv[0]

    li_w3 = np.concatenate([g["li_w"], g["li_b"][None, :]], axis=0).astype(f32)
    ci_w3 = np.concatenate([g["ci_w"], g["ci_b"][None, :]], axis=0).astype(f32)

    # adjacency per graph from edge_index (direction-robust)
    ei = g["edge_index"].astype(np.int64)
    src, dst = ei[0], ei[1]
    src_g, dst_g = src // NPG, dst // NPG
    assert np.all(src_g == dst_g), "edges must be graph-local"
    src_l, dst_l = src % NPG, dst % NPG
    s_lit, d_lit = src_l < NL, dst_l < NL
    A_in_c = np.zeros((B, NC, NL), f32)   # clause <- literal edges
    m = (~d_lit) & s_lit
    np.add.at(A_in_c, (dst_g[m], dst_l[m] - NL, src_l[m]), 1.0)
    A_in_l = np.zeros((B, NL, NC), f32)   # literal <- clause edges
    m = d_lit & (~s_lit)
    np.add.at(A_in_l, (dst_g[m], dst_l[m], src_l[m] - NL), 1.0)
    deg_l = A_in_l.sum(axis=2)            # [B, NL]

    x = g["x"].astype(f32).reshape(B, NPG, 2)
    ones = np.ones((B, NPG, 1), f32)
    x3 = np.concatenate([x, ones], axis=2)        # [B, NPG, 3]

    shared = dict(
        wc_a=wc_a, wc_b=wc_b, wc_1=wc_1, wl_a=wl_a, wl_b=wl_b,
        w_lh_dup=w_lh_dup, w_cl2_dup=w_cl2_dup, wv_dup=wv_dup,
        li_w3=li_w3, ci_w3=ci_w3, bias_dup=bias_dup,
    )
    in_maps = []
    for c in range(NCORES):
        gs = slice(c * GPC, (c + 1) * GPC)
        x3c = x3[gs]                               # [GPC, NPG, 3]
        xt_lit = np.ascontiguousarray(
            x3c[:, :NL].transpose(2, 0, 1).reshape(3, GPC * NL))
        xt_cl = np.ascontiguousarray(
            x3c[:, NL:].transpose(2, 0, 1).reshape(3, GPC * NC))
        # dqq[x]: rows 0:64 = q_x (x) (deg_g0+1) + cbias_l_x ; rows 64:128 g1
        dqq = np.zeros((128, 4 * NL), f32)
        for x_ in range(4):
            qx = q_l[x_ * H:(x_ + 1) * H]
            cbx = cbias_l[x_ * H:(x_ + 1) * H]
            for gg in range(GPC):
                d1 = deg_l[c * GPC + gg] + 1.0
                dqq[gg * 64:(gg + 1) * 64, x_ * NL:(x_ + 1) * NL] = (
                    np.outer(qx, d1) + cbx[:, None])
        # pre-chunk adjacency into full-128-row K-chunks; the final chunk
        # overlaps the previous one with its overlap rows zeroed
        atc = np.zeros((GPC, 7, 128, NC), f32)
        ac = np.zeros((GPC, 4, 128, NL), f32)
        for gg in range(GPC):
            at_full = A_in_c[c * GPC + gg].T       # [NL, NC]
            a_full = A_in_l[c * GPC + gg].T        # [NC, NL]
            for k in range(6):
                atc[gg, k] = at_full[128 * k:128 * (k + 1)]
            atc[gg, 6, 128 - (NL - 768):] = at_full[768:]
            for k in range(3):
                ac[gg, k] = a_full[128 * k:128 * (k + 1)]
            ac[gg, 3, 128 - (NC - 384):] = a_full[384:]
        in_maps.append(dict(
            xt_lit=xt_lit, xt_cl=xt_cl, at_rm=atc, a_rm=ac,
            dqq=dqq, **shared,
        ))
    return in_maps


_LAST_RESULTS = {}


def kernel(**inputs):
    from concourse.bass_utils import run_bass_kernel_spmd

    in_maps = _fold_and_shard(inputs)
    if "nc" not in _PROGRAM_CACHE:
        _PROGRAM_CACHE["nc"] = _build_program()
    nc = _PROGRAM_CACHE["nc"]
    res = run_bass_kernel_spmd(nc, in_maps, core_ids=list(range(NCORES)))
    _LAST_RESULTS["res"] = res
    out = np.zeros((N, 1), np.float32)
    for c in range(NCORES):
        vote = res.results[c]["vote"].reshape(GPC, NL)
        for g in range(GPC):
            base = (c * GPC + g) * NPG
            out[base:base + NL, 0] = vote[g]
    return out



# revision 10
# speedup vs baseline: 1.1664x; 1.1664x over previous
"""NeuroSAT GNN message passing on 8 Trainium2 NeuronCores — v2.

Graph-data-parallel: 2 graphs/core, zero collectives. Structure vs v1:

  * Row-major (node-major) LSTM gates: out[node-block, 256] = x_fm.T @ W.
    The gate stationary IS the feature-major state (no pre-transpose), the
    four gates land column-adjacent so the whole LSTM nonlinearity batch
    runs as ONE sigmoid per 1024-col psum tile (tanh via 2*sig(2x)-1, the
    g-gate pre-scaled x2 into W and bias on the host).
  * Aggregation in "orientation B": out[node-block, 64] accumulates
    A^T-chunk-stationary x h_rm-moving matmuls. Since the cost of a matmul
    is (moving free size) x cycles/row, keeping N=64 on the feature dim and
    the 800/440-node dims on M/K cuts agg PE time ~2x vs feature-major.
  * Per-gate bias (incl. the degree-dependent literal bias) is preloaded
    into PSUM by an ACT/DVE copy; gate matmuls accumulate on top
    (start=False, skip_group_check).
  * flip_perm is a per-graph half-swap; with 100-wide literal blocks the
    flip of a block is another block -> read the gate stationary at
    shifted columns, no data movement.
  * Tail iterations (t >= TR=19): gate matmuls + vote in fp32r (tf32,
    4x faster at N>=256) and aggregations with bf16 messages against the
    exact bf16 0/1 adjacency (orientation A, feature-major out). Numpy
    error study across all 16 graphs: worst rel err 0.0094 < 2e-2 gate.
"""

import numpy as np

H = 64
ITERS = 24
B, NV, NC, K = 16, 400, 440, 12
NL = 2 * NV                  # literals/graph = 800
NPG = NL + NC                # nodes/graph = 1240
N = B * NPG                  # 19840
NCORES = 8
GPC = B // NCORES            # graphs per core = 2
LB, NLB = 100, 8             # literal block size/count (800 = 8*100)
CB, NCB = 110, 4             # clause block size/count (440 = 4*110)
TR = 19                      # first low-precision tail iteration
TR_AGG = TR                  # first bf16-aggregation iteration
TR_GATE = TR                 # first fp32r-gate iteration (also vote dtype)

_PROGRAM_CACHE = {}


def _build_program():
    from contextlib import ExitStack

    import concourse.bacc as bacc
    import concourse.mybir as mybir
    from concourse.masks import make_identity
    from concourse.tile import TileContext

    F32 = mybir.dt.float32
    F32R = mybir.dt.float32r
    BF16 = mybir.dt.bfloat16
    SIG = mybir.ActivationFunctionType.Sigmoid
    IDENT = mybir.ActivationFunctionType.Identity
    MULT = mybir.AluOpType.mult
    SUB = mybir.AluOpType.subtract

    nc = bacc.Bacc(
        "TRN2", target_bir_lowering=False, debug=False, num_devices=NCORES
    )

    # ---- DRAM I/O ----
    d_atc = nc.dram_tensor("atc", [GPC, NLB, LB, NC], F32, kind="ExternalInput")
    d_atc_b = nc.dram_tensor("atc_b", [GPC, NLB, LB, NC], BF16, kind="ExternalInput")
    d_alc = nc.dram_tensor("alc", [GPC, NCB, CB, NL], F32, kind="ExternalInput")
    d_alc_b = nc.dram_tensor("alc_b", [GPC, NCB, CB, NL], BF16, kind="ExternalInput")
    d_wc1 = nc.dram_tensor("wc1", [128, 256], F32, kind="ExternalInput")
    d_wc = nc.dram_tensor("wc", [128, 256], F32, kind="ExternalInput")
    d_wc_r = nc.dram_tensor("wc_r", [128, 256], F32R, kind="ExternalInput")
    d_wl1 = nc.dram_tensor("wl1", [128, 256], F32, kind="ExternalInput")
    d_wl = nc.dram_tensor("wl", [128, 256], F32, kind="ExternalInput")
    d_wl_r = nc.dram_tensor("wl_r", [128, 256], F32R, kind="ExternalInput")
    d_wf = nc.dram_tensor("wf", [128, 256], F32, kind="ExternalInput")
    d_wf_r = nc.dram_tensor("wf_r", [128, 256], F32R, kind="ExternalInput")
    d_wv_r = nc.dram_tensor("wv_r", [128, 1], F32R, kind="ExternalInput")
    d_bias_cl = nc.dram_tensor("bias_cl", [CB, 1024], F32, kind="ExternalInput")
    d_bias_lit = nc.dram_tensor("bias_lit", [GPC, LB, 2048], F32, kind="ExternalInput")
    d_bv = nc.dram_tensor("bv", [1, 1], F32, kind="ExternalInput")
    d_xt_lit = nc.dram_tensor("xt_lit", [3, GPC * NL], F32, kind="ExternalInput")
    d_xt_cl = nc.dram_tensor("xt_cl", [3, GPC * NC], F32, kind="ExternalInput")
    d_liw = nc.dram_tensor("li_w3", [3, H], F32, kind="ExternalInput")
    d_ciw = nc.dram_tensor("ci_w3", [3, H], F32, kind="ExternalInput")
    d_out = nc.dram_tensor("vote", [1, GPC * NL], F32, kind="ExternalOutput")

    with TileContext(nc) as tc, ExitStack() as ctx:
        const = ctx.enter_context(tc.tile_pool(name="const", bufs=1))
        state = ctx.enter_context(tc.tile_pool(name="state", bufs=2))
        work = ctx.enter_context(tc.tile_pool(name="work", bufs=2))
        ps = ctx.enter_context(tc.tile_pool(name="ps", bufs=1, space="PSUM"))

        ident = const.tile([128, 128], F32, name="ident")
        make_identity(nc, ident)

        def load(dram, shape, nm, dt=F32):
            t = const.tile(shape, dt, name=nm)
            nc.sync.dma_start(out=t[:, :], in_=dram[:, :])
            return t

        atc = const.tile([LB, GPC * NLB * NC], F32, name="atc_sb")
        atc_b = const.tile([LB, GPC * NLB * NC], BF16, name="atc_b_sb")
        for g in range(GPC):
            for k in range(NLB):
                c0 = (g * NLB + k) * NC
                nc.sync.dma_start(out=atc[:, c0:c0 + NC], in_=d_atc[g, k])
                nc.sync.dma_start(out=atc_b[:, c0:c0 + NC], in_=d_atc_b[g, k])
        alc = const.tile([CB, GPC * NCB * NL], F32, name="alc_sb")
        alc_b = const.tile([CB, GPC * NCB * NL], BF16, name="alc_b_sb")
        for g in range(GPC):
            for j in range(NCB):
                c0 = (g * NCB + j) * NL
                nc.sync.dma_start(out=alc[:, c0:c0 + NL], in_=d_alc[g, j])
                nc.sync.dma_start(out=alc_b[:, c0:c0 + NL], in_=d_alc_b[g, j])

        wc1 = load(d_wc1, [128, 256], "wc1_sb")
        wc = load(d_wc, [128, 256], "wc_sb")
        wc_r = load(d_wc_r, [128, 256], "wc_r_sb", F32R)
        wl1 = load(d_wl1, [128, 256], "wl1_sb")
        wl = load(d_wl, [128, 256], "wl_sb")
        wl_r = load(d_wl_r, [128, 256], "wl_r_sb", F32R)
        wf = load(d_wf, [128, 256], "wf_sb")
        wf_r = load(d_wf_r, [128, 256], "wf_r_sb", F32R)
        wv_r = load(d_wv_r, [128, 1], "wv_r_sb", F32R)
        wv_f = const.tile([128, 1], F32, name="wv_f_sb")
        nc.vector.tensor_copy(wv_f[:, :], wv_r[:, :])
        bias_cl = load(d_bias_cl, [CB, 1024], "bias_cl_sb")
        bias_lit = const.tile([LB, GPC * 2048], F32, name="bias_lit_sb")
        for g in range(GPC):
            nc.sync.dma_start(
                out=bias_lit[:, g * 2048:(g + 1) * 2048], in_=d_bias_lit[g])
        bv = load(d_bv, [1, 1], "bv_sb")
        xt_lit = load(d_xt_lit, [3, GPC * NL], "xt_lit_sb")
        xt_cl = load(d_xt_cl, [3, GPC * NC], "xt_cl_sb")
        li_w3 = load(d_liw, [3, H], "li_w3_sb")
        ci_w3 = load(d_ciw, [3, H], "ci_w3_sb")

        def MM(*a, **kw):
            kw.setdefault("skip_group_check", True)
            return nc.tensor.matmul(*a, **kw)

        # engine rotation for psum-touching copies: ACT/DVE only
        # (GPSIMD instructions cannot access PSUM on trn2)
        _cyc = [0]
        COPY_ENGINES = (nc.scalar.copy, nc.vector.tensor_copy)

        def rcopy(out, in_):
            f = COPY_ENGINES[_cyc[0] % 2]
            _cyc[0] += 1
            f(out, in_)

        # preloads may only come from ACT or DVE (gpsimd->psum is broken)
        _pcyc = [0]

        def preload(out, in_):
            f = (nc.scalar.copy, nc.vector.tensor_copy)[_pcyc[0] % 2]
            _pcyc[0] += 1
            f(out, in_)

        def b3(ap, b):
            return ap.rearrange("p (b z) -> p b z", b=b)

        # ---------- state transposes: h_rm -> fm halves of next x tiles ----
        def make_x(t, g, lh_rm, ch_rm, with_cl=True):
            """Allocate xl/xc for iteration t and fill rows 64:128 with the
            transposed h state. Rows 0:64 are filled by the agg phases."""
            dt = F32R if t >= TR_GATE else F32
            xl = state.tile([128, NL], dt, tag=f"xl{g}", name=f"xl{g}_{t}")
            ptr0 = ps.tile([64, 400], F32, tag="pt", bufs=2, name=f"ptl0_{t}_{g}")
            ptr1 = ps.tile([64, 400], F32, tag="pt", bufs=2, name=f"ptl1_{t}_{g}")
            for mb in range(NLB):
                dst = (ptr0, ptr1)[mb // 4]
                nc.tensor.transpose(
                    dst[:, (mb % 4) * LB:(mb % 4 + 1) * LB],
                    lh_rm[:, mb * H:(mb + 1) * H],
                    ident[0:LB, 0:LB],
                )
            rcopy(xl[64:128, 0:400], ptr0[:, :])
            rcopy(xl[64:128, 400:800], ptr1[:, :])
            if not with_cl:
                return xl, None
            xc = state.tile([128, NC], dt, tag=f"xc{g}", name=f"xc{g}_{t}")
            ptc = ps.tile([64, NC], F32, tag="pt", bufs=2, name=f"ptc_{t}_{g}")
            for mb in range(NCB):
                nc.tensor.transpose(
                    ptc[:, mb * CB:(mb + 1) * CB],
                    ch_rm[:, mb * H:(mb + 1) * H],
                    ident[0:CB, 0:CB],
                )
            rcopy(xc[64:128, :], ptc[:, :])
            return xl, xc

        # ---------- initial states (rm) ----------
        lh_cur, ch_cur = {}, {}
        lh_b_cur, ch_b_cur = {}, {}
        lc_cur, cc_cur = {g: None for g in range(GPC)}, {g: None for g in range(GPC)}
        for g in range(GPC):
            pl = ps.tile([LB, 512], F32, tag="pa", bufs=2, name=f"inl_{g}")
            for mb in range(NLB):
                MM(pl[:, mb * H:(mb + 1) * H],
                   xt_lit[0:3, g * NL + mb * LB:g * NL + (mb + 1) * LB],
                   li_w3[0:3, :], start=True, stop=True)
            lh0 = state.tile([LB, 512], F32, tag=f"lh{g}", name=f"lh{g}_0")
            nc.scalar.copy(lh0[:, :], pl[:, :])
            pc = ps.tile([CB, 256], F32, tag="pa", bufs=2, name=f"inc_{g}")
            for mb in range(NCB):
                MM(pc[:, mb * H:(mb + 1) * H],
                   xt_cl[0:3, g * NC + mb * CB:g * NC + (mb + 1) * CB],
                   ci_w3[0:3, :], start=True, stop=True)
            ch0 = state.tile([CB, 256], F32, tag=f"ch{g}", name=f"ch{g}_0")
            nc.scalar.copy(ch0[:, :], pc[:, :])
            lh_cur[g], ch_cur[g] = lh0, ch0

        xl_cur, xc_cur = {}, {}
        for g in range(GPC):
            xl_cur[g], xc_cur[g] = make_x(1, g, lh_cur[g], ch_cur[g])

        # ---------- iterations ----------
        for t in range(1, ITERS):
            tail_a = t >= TR_AGG
            tail_g = t >= TR_GATE

            # -- phase A: clause agg -> xc[0:64] --
            for g in range(GPC):
                if not tail_a:
                    pa = ps.tile([CB, 256], F32, tag="pa", bufs=2,
                                 name=f"pac_{t}_{g}")
                    for mb in range(NCB):
                        for k in range(NLB):
                            c0 = (g * NLB + k) * NC + mb * CB
                            MM(pa[:, mb * H:(mb + 1) * H],
                               atc[:, c0:c0 + CB],
                               lh_cur[g][:, k * H:(k + 1) * H],
                               start=(k == 0), stop=(k == NLB - 1))
                    a_sb = work.tile([CB, 256], F32, tag="agsb",
                                     name=f"asb_{t}_{g}")
                    rcopy(a_sb[:, :], pa[:, :])
                    pat = ps.tile([64, NC], F32, tag="pt", bufs=2,
                                  name=f"pat_{t}_{g}")
                    for mb in range(NCB):
                        nc.tensor.transpose(
                            pat[:, mb * CB:(mb + 1) * CB],
                            a_sb[:, mb * H:(mb + 1) * H],
                            ident[0:CB, 0:CB],
                        )
                    rcopy(xc_cur[g][0:64, :], pat[:, :])
                else:
                    pfa = ps.tile([64, NC], F32, tag="pt", bufs=2,
                                  name=f"pfa_{t}_{g}")
                    for k in range(NLB):
                        c0 = (g * NLB + k) * NC
                        MM(pfa[:, :],
                           lh_b_cur[g][:, k * H:(k + 1) * H],
                           atc_b[:, c0:c0 + NC],
                           start=(k == 0), stop=(k == NLB - 1))
                    rcopy(xc_cur[g][0:64, :], pfa[:, :])

            # -- phase B: clause gates + LSTM --
            for g in range(GPC):
                pgc = ps.tile([CB, 1024], F32, tag="pg", bufs=2,
                              name=f"pgc_{t}_{g}")
                preload(pgc[:, :], bias_cl[:, :])
                W = wc1 if t == 1 else (wc_r if tail_g else wc)
                for mb in range(NCB):
                    MM(pgc[:, mb * 256:(mb + 1) * 256],
                       xc_cur[g][:, mb * CB:(mb + 1) * CB],
                       W[:, :], start=False, stop=True)
                S = work.tile([CB, 1024], F32, tag=f"Sc{g}", bufs=1,
                              name=f"Sc_{t}_{g}")
                nc.scalar.activation(S[:, :], pgc[:, :], SIG)
                si = b3(S[:, :], 4)[:, :, 0:64]
                sf = b3(S[:, :], 4)[:, :, 64:128]
                sg = b3(S[:, :], 4)[:, :, 128:192]
                so = b3(S[:, :], 4)[:, :, 192:256]
                t1 = work.tile([CB, 256], F32, tag="t1", name=f"ct1_{t}_{g}")
                t1v = b3(t1[:, :], 4)
                nc.gpsimd.tensor_mul(t1v, si, sg)
                cc_new = state.tile([CB, 256], F32, tag=f"cc{g}",
                                    name=f"cc{g}_{t}")
                ccv = b3(cc_new[:, :], 4)
                if t == 1:
                    nc.vector.scalar_tensor_tensor(
                        ccv, t1v, 2.0, si, op0=MULT, op1=SUB)
                else:
                    nc.vector.scalar_tensor_tensor(
                        t1v, t1v, 2.0, si, op0=MULT, op1=SUB)
                    nc.gpsimd.tensor_mul(sf, sf, b3(cc_cur[g][:, :], 4))
                    nc.vector.tensor_add(ccv, t1v, sf)
                nc.scalar.activation(sg, ccv, SIG, scale=2.0)
                nc.gpsimd.tensor_mul(t1v, so, sg)
                ch_new = state.tile([CB, 256], F32, tag=f"ch{g}",
                                    name=f"ch{g}_{t}")
                nc.vector.scalar_tensor_tensor(
                    b3(ch_new[:, :], 4), t1v, 2.0, so, op0=MULT, op1=SUB)
                cc_cur[g], ch_cur[g] = cc_new, ch_new
                if tail_a:
                    chb = state.tile([CB, 256], BF16, tag=f"chb{g}", bufs=1,
                                     name=f"chb{g}_{t}")
                    nc.gpsimd.tensor_copy(chb[:, :], ch_new[:, :])
                    ch_b_cur[g] = chb

            # -- phase C: lit agg -> xl[0:64] --
            for g in range(GPC):
                if not tail_a:
                    pa = ps.tile([LB, 512], F32, tag="pa", bufs=2,
                                 name=f"pal_{t}_{g}")
                    for mb in range(NLB):
                        for j in range(NCB):
                            c0 = (g * NCB + j) * NL + mb * LB
                            MM(pa[:, mb * H:(mb + 1) * H],
                               alc[:, c0:c0 + LB],
                               ch_cur[g][:, j * H:(j + 1) * H],
                               start=(j == 0), stop=(j == NCB - 1))
                    c_sb = work.tile([LB, 512], F32, tag="agsb",
                                     name=f"csb_{t}_{g}")
                    rcopy(c_sb[:, :], pa[:, :])
                    ptl0 = ps.tile([64, 400], F32, tag="pt", bufs=2,
                                   name=f"pbl0_{t}_{g}")
                    ptl1 = ps.tile([64, 400], F32, tag="pt", bufs=2,
                                   name=f"pbl1_{t}_{g}")
                    for mb in range(NLB):
                        dst = (ptl0, ptl1)[mb // 4]
                        nc.tensor.transpose(
                            dst[:, (mb % 4) * LB:(mb % 4 + 1) * LB],
                            c_sb[:, mb * H:(mb + 1) * H],
                            ident[0:LB, 0:LB],
                        )
                    rcopy(xl_cur[g][0:64, 0:400], ptl0[:, :])
                    rcopy(xl_cur[g][0:64, 400:800], ptl1[:, :])
                else:
                    for hf in range(2):
                        pfl = ps.tile([64, 400], F32, tag="pt", bufs=2,
                                      name=f"pfl_{t}_{g}_{hf}")
                        for j in range(NCB):
                            c0 = (g * NCB + j) * NL + hf * 400
                            MM(pfl[:, :],
                               ch_b_cur[g][:, j * H:(j + 1) * H],
                               alc_b[:, c0:c0 + 400],
                               start=(j == 0), stop=(j == NCB - 1))
                        rcopy(xl_cur[g][0:64, hf * 400:(hf + 1) * 400],
                              pfl[:, :])

            # -- phase D: lit gates + LSTM --
            for g in range(GPC):
                S = work.tile([LB, 2048], F32, tag=f"Sl{g}", bufs=1,
                              name=f"Sl_{t}_{g}")
                WL = wl1 if t == 1 else (wl_r if tail_g else wl)
                WF = wf_r if tail_g else wf
                for hf in range(2):
                    pgl = ps.tile([LB, 1024], F32, tag="pg", bufs=2,
                                  name=f"pgl_{t}_{g}_{hf}")
                    preload(pgl[:, :],
                            bias_lit[:, g * 2048 + hf * 1024:
                                     g * 2048 + (hf + 1) * 1024])
                    for mbl in range(4):
                        mb = hf * 4 + mbl
                        MM(pgl[:, mbl * 256:(mbl + 1) * 256],
                           xl_cur[g][:, mb * LB:(mb + 1) * LB],
                           WL[:, :], start=False, stop=False)
                        fb = ((mb + 4) % NLB) * LB
                        MM(pgl[:, mbl * 256:(mbl + 1) * 256],
                           xl_cur[g][64:128, fb:fb + LB],
                           WF[64:128, :], start=False, stop=True)
                    nc.scalar.activation(
                        S[:, hf * 1024:(hf + 1) * 1024], pgl[:, :], SIG)
                si = b3(S[:, :], 8)[:, :, 0:64]
                sf = b3(S[:, :], 8)[:, :, 64:128]
                sg = b3(S[:, :], 8)[:, :, 128:192]
                so = b3(S[:, :], 8)[:, :, 192:256]
                t1 = work.tile([LB, 512], F32, tag="t1", name=f"lt1_{t}_{g}")
                t1v = b3(t1[:, :], 8)
                nc.gpsimd.tensor_mul(t1v, si, sg)
                lc_new = state.tile([LB, 512], F32, tag=f"lc{g}",
                                    name=f"lc{g}_{t}")
                lcv = b3(lc_new[:, :], 8)
                if t == 1:
                    nc.vector.scalar_tensor_tensor(
                        lcv, t1v, 2.0, si, op0=MULT, op1=SUB)
                else:
                    nc.vector.scalar_tensor_tensor(
                        t1v, t1v, 2.0, si, op0=MULT, op1=SUB)
                    nc.gpsimd.tensor_mul(sf, sf, b3(lc_cur[g][:, :], 8))
                    nc.vector.tensor_add(lcv, t1v, sf)
                nc.scalar.activation(sg, lcv, SIG, scale=2.0)
                nc.gpsimd.tensor_mul(t1v, so, sg)
                lh_new = state.tile([LB, 512], F32, tag=f"lh{g}",
                                    name=f"lh{g}_{t}")
                nc.vector.scalar_tensor_tensor(
                    b3(lh_new[:, :], 8), t1v, 2.0, so, op0=MULT, op1=SUB)
                lc_cur[g], lh_cur[g] = lc_new, lh_new
                if t + 1 >= TR_AGG:
                    lhb = state.tile([LB, 512], BF16, tag=f"lhb{g}", bufs=1,
                                     name=f"lhb{g}_{t}")
                    nc.gpsimd.tensor_copy(lhb[:, :], lh_new[:, :])
                    lh_b_cur[g] = lhb

            # -- phase E: transposes for next iteration --
            for g in range(GPC):
                xl_cur[g], xc_cur[g] = make_x(
                    t + 1, g, lh_cur[g], ch_cur[g], with_cl=(t < ITERS - 1))

        # ---- vote head: xl_24[64:128] holds lh fm (F32R) ----
        wv_use = wv_r if ITERS >= TR_GATE else wv_f
        for g in range(GPC):
            for hf in range(2):
                pv = ps.tile([1, 400], F32, tag="pa", bufs=2,
                             name=f"pv_{g}_{hf}")
                MM(pv[:, :], wv_use[64:128, 0:1],
                   xl_cur[g][64:128, hf * 400:(hf + 1) * 400],
                   start=True, stop=True)
                vo = work.tile([1, 400], F32, tag="vote", name=f"vo_{g}_{hf}")
                nc.scalar.activation(vo[0:1, :], pv[:, :], IDENT,
                                     bias=bv[0:1, 0:1])
                nc.sync.dma_start(
                    out=d_out[0:1, g * NL + hf * 400:g * NL + (hf + 1) * 400],
                    in_=vo[0:1, :])

    nc.compile()
    return nc


def _fold_and_shard(inputs):
    """Host-side preprocessing: fold weights, build adjacency, shard by graph."""
    f32 = np.float32
    import ml_dtypes
    bf16 = ml_dtypes.bfloat16
    g = {k: np.asarray(v) for k, v in inputs.items()}

    def collapse(w1, b1, w2, b2, w3, b3):
        return w1 @ w2 @ w3, ((b1 @ w2) + b2) @ w3 + b3

    Wl, bl = collapse(g["lm1_w"], g["lm1_b"], g["lm2_w"], g["lm2_b"],
                      g["lm3_w"], g["lm3_b"])
    Wc, bc = collapse(g["cm1_w"], g["cm1_b"], g["cm2_w"], g["cm2_b"],
                      g["cm3_w"], g["cm3_b"])
    Wv, bv = collapse(g["lv1_w"], g["lv1_b"], g["lv2_w"], g["lv2_b"],
                      g["lv3_w"], g["lv3_b"])

    cu_wih, lu_wih = g["cu_wih"], g["lu_wih"]
    w_lc = (Wl @ cu_wih).astype(f32)                 # agg_c -> clause gates
    w_ch = (w_lc + g["cu_whh"]).astype(f32)          # t>=2 merged recurrent
    cbias_c = ((K + 1) * (bl @ cu_wih) + g["cu_bih"] + g["cu_bhh"]).astype(f32)
    wih_a = lu_wih[0:H].astype(f32)                  # flip -> lit gates
    w_cl2 = (Wc @ lu_wih[H:2 * H]).astype(f32)       # agg_l -> lit gates
    w_lh = (w_cl2 + g["lu_whh"]).astype(f32)         # t>=2 merged recurrent
    q_l = (bc @ lu_wih[H:2 * H]).astype(f32)         # [256]
    cbias_l = (g["lu_bih"] + g["lu_bhh"]).astype(f32)

    def gscale(w):
        """double the g-gate columns (128:192) so one sigmoid serves all
        four gates (tanh(x) = 2*sig(2x)-1)."""
        w = w.copy()
        w[..., 128:192] *= 2.0
        return w

    vs = np.vstack
    wc1 = gscale(vs([w_lc, w_lc]))
    wc_m = gscale(vs([w_lc, w_ch]))
    wl1 = gscale(vs([w_cl2, w_cl2]))
    wl_m = gscale(vs([w_cl2, w_lh]))
    wf_m = gscale(wih_a)
    cbias_c_s = gscale(cbias_c)
    bias_cl = np.tile(cbias_c_s[None, :], (CB, 4)).astype(f32)  # [110, 1024]

    li_w3 = np.concatenate([g["li_w"], g["li_b"][None, :]], axis=0).astype(f32)
    ci_w3 = np.concatenate([g["ci_w"], g["ci_b"][None, :]], axis=0).astype(f32)

    # adjacency per graph from edge_index (direction-robust)
    ei = g["edge_index"].astype(np.int64)
    src, dst = ei[0], ei[1]
    src_g, dst_g = src // NPG, dst // NPG
    assert np.all(src_g == dst_g), "edges must be graph-local"
    src_l, dst_l = src % NPG, dst % NPG
    s_lit, d_lit = src_l < NL, dst_l < NL
    A_in_c = np.zeros((B, NC, NL), f32)   # clause <- literal edges
    m = (~d_lit) & s_lit
    np.add.at(A_in_c, (dst_g[m], dst_l[m] - NL, src_l[m]), 1.0)
    A_in_l = np.zeros((B, NL, NC), f32)   # literal <- clause edges
    m = d_lit & (~s_lit)
    np.add.at(A_in_l, (dst_g[m], dst_l[m], src_l[m] - NL), 1.0)
    deg_l = A_in_l.sum(axis=2)            # [B, NL]

    x = g["x"].astype(f32).reshape(B, NPG, 2)
    ones = np.ones((B, NPG, 1), f32)
    x3 = np.concatenate([x, ones], axis=2)        # [B, NPG, 3]

    q_s = gscale(q_l)
    cb_s = gscale(cbias_l)

    wf128 = np.vstack([wf_m, wf_m]).astype(f32)          # rows 64:128 used
    wv128 = np.vstack([Wv, Wv]).astype(f32)
    shared = dict(
        wc1=wc1, wc=wc_m, wc_r=wc_m, wl1=wl1, wl=wl_m, wl_r=wl_m,
        wf=wf128, wf_r=wf128, wv_r=wv128,
        bias_cl=bias_cl, bv=np.array([[bv[0]]], f32),
        li_w3=li_w3, ci_w3=ci_w3,
    )
    in_maps = []
    for c in range(NCORES):
        atc = np.zeros((GPC, NLB, LB, NC), f32)
        alc = np.zeros((GPC, NCB, CB, NL), f32)
        bias_lit = np.zeros((GPC, LB, 2048), f32)
        for gg in range(GPC):
            gid = c * GPC + gg
            at_full = A_in_c[gid].T              # [NL, NC]
            al_full = A_in_l[gid].T              # [NC, NL]
            for k in range(NLB):
                atc[gg, k] = at_full[LB * k:LB * (k + 1)]
            for j in range(NCB):
                alc[gg, j] = al_full[CB * j:CB * (j + 1)]
            # bias_lit[g][p, mb*256 + j] = q_s[j]*(deg[mb*100+p]+1) + cb_s[j]
            d1 = deg_l[gid] + 1.0                # [800]
            bl_g = (d1[:, None] * q_s[None, :] + cb_s[None, :])  # [800, 256]
            bias_lit[gg] = bl_g.reshape(NLB, LB, 256).transpose(1, 0, 2).reshape(LB, 2048)
        gs = slice(c * GPC, (c + 1) * GPC)
        x3c = x3[gs]                               # [GPC, NPG, 3]
        xt_lit = np.ascontiguousarray(
            x3c[:, :NL].transpose(2, 0, 1).reshape(3, GPC * NL))
        xt_cl = np.ascontiguousarray(
            x3c[:, NL:].transpose(2, 0, 1).reshape(3, GPC * NC))
        in_maps.append(dict(
            atc=atc, atc_b=atc.astype(bf16), alc=alc, alc_b=alc.astype(bf16),
            bias_lit=bias_lit, xt_lit=xt_lit, xt_cl=xt_cl, **shared,
        ))
    return in_maps


_LAST_RESULTS = {}


def kernel(**inputs):
    from concourse.bass_utils import run_bass_kernel_spmd

    in_maps = _fold_and_shard(inputs)
    if "nc" not in _PROGRAM_CACHE:
        _PROGRAM_CACHE["nc"] = _build_program()
    nc = _PROGRAM_CACHE["nc"]
    res = run_bass_kernel_spmd(nc, in_maps, core_ids=list(range(NCORES)))
    _LAST_RESULTS["res"] = res
    out = np.zeros((N, 1), np.float32)
    for c in range(NCORES):
        vote = res.results[c]["vote"].reshape(GPC, NL)
        for g in range(GPC):
            base = (c * GPC + g) * NPG
            out[base:base + NL, 0] = vote[g]
    return out


# revision 31
# speedup vs baseline: 1.6119x; 1.3819x over previous
"""NeuroSAT GNN message passing on 8 Trainium2 NeuronCores — v2.

Graph-data-parallel: 2 graphs/core, zero collectives. Structure vs v1:

  * Row-major (node-major) LSTM gates: out[node-block, 256] = x_fm.T @ W.
    The gate stationary IS the feature-major state (no pre-transpose), the
    four gates land column-adjacent so the whole LSTM nonlinearity batch
    runs as ONE sigmoid per 1024-col psum tile (tanh via 2*sig(2x)-1, the
    g-gate pre-scaled x2 into W and bias on the host).
  * Aggregation in "orientation B": out[node-block, 64] accumulates
    A^T-chunk-stationary x h_rm-moving matmuls. Since the cost of a matmul
    is (moving free size) x cycles/row, keeping N=64 on the feature dim and
    the 800/440-node dims on M/K cuts agg PE time ~2x vs feature-major.
  * Per-gate bias (incl. the degree-dependent literal bias) is preloaded
    into PSUM by an ACT/DVE copy; gate matmuls accumulate on top
    (start=False, skip_group_check).
  * flip_perm is a per-graph half-swap; with 100-wide literal blocks the
    flip of a block is another block -> read the gate stationary at
    shifted columns, no data movement.
  * Tail iterations (t >= TR=19): gate matmuls + vote in fp32r (tf32,
    4x faster at N>=256) and aggregations with bf16 messages against the
    exact bf16 0/1 adjacency (orientation A, feature-major out). Numpy
    error study across all 16 graphs: worst rel err 0.0094 < 2e-2 gate.
"""

import numpy as np

H = 64
ITERS = 24
B, NV, NC, K = 16, 400, 440, 12
NL = 2 * NV                  # literals/graph = 800
NPG = NL + NC                # nodes/graph = 1240
N = B * NPG                  # 19840
NCORES = 8
GPC = B // NCORES            # graphs per core = 2
LB, NLB = 100, 8             # literal block size/count (800 = 8*100)
CB, NCB = 110, 4             # clause block size/count (440 = 4*110)
TR = 19                      # first low-precision tail iteration
DEBUG_DUMP = False           # add per-iteration state dumps (debug builds)
TR_AGG = TR                  # first bf16-aggregation iteration
TR_GATE = TR                 # first fp32r-gate iteration (also vote dtype)

_PROGRAM_CACHE = {}


def _build_program():
    from contextlib import ExitStack

    import concourse.bacc as bacc
    import concourse.mybir as mybir
    from concourse.masks import make_identity
    from concourse.tile import TileContext, add_dep_helper

    F32 = mybir.dt.float32
    F32R = mybir.dt.float32r
    BF16 = mybir.dt.bfloat16
    SIG = mybir.ActivationFunctionType.Sigmoid
    IDENT = mybir.ActivationFunctionType.Identity
    MULT = mybir.AluOpType.mult
    SUB = mybir.AluOpType.subtract

    nc = bacc.Bacc(
        "TRN2", target_bir_lowering=False, debug=False, num_devices=NCORES
    )

    # ---- DRAM I/O ----
    d_atc = nc.dram_tensor("atc", [GPC, NLB, LB, NC], F32, kind="ExternalInput")
    d_atc_b = nc.dram_tensor("atc_b", [GPC, NLB, LB, NC], BF16, kind="ExternalInput")
    d_alc = nc.dram_tensor("alc", [GPC, NCB, CB, NL], F32, kind="ExternalInput")
    d_alc_b = nc.dram_tensor("alc_b", [GPC, NCB, CB, NL], BF16, kind="ExternalInput")
    d_wc1 = nc.dram_tensor("wc1", [128, 256], F32, kind="ExternalInput")
    d_wc = nc.dram_tensor("wc", [128, 256], F32, kind="ExternalInput")
    d_wc_r = nc.dram_tensor("wc_r", [128, 256], F32R, kind="ExternalInput")
    d_wl1 = nc.dram_tensor("wl1", [128, 256], F32, kind="ExternalInput")
    d_wl = nc.dram_tensor("wl", [128, 256], F32, kind="ExternalInput")
    d_wl_r = nc.dram_tensor("wl_r", [128, 256], F32R, kind="ExternalInput")
    d_wf = nc.dram_tensor("wf", [128, 256], F32, kind="ExternalInput")
    d_wf_r = nc.dram_tensor("wf_r", [128, 256], F32R, kind="ExternalInput")
    d_wv_r = nc.dram_tensor("wv_r", [128, 1], F32R, kind="ExternalInput")
    d_bias_cl = nc.dram_tensor("bias_cl", [CB, 1024], F32, kind="ExternalInput")
    d_bias_lit = nc.dram_tensor("bias_lit", [GPC, LB, 2048], F32, kind="ExternalInput")
    d_bv = nc.dram_tensor("bv", [1, 1], F32, kind="ExternalInput")
    d_xt_lit = nc.dram_tensor("xt_lit", [3, GPC * NL], F32, kind="ExternalInput")
    d_xt_cl = nc.dram_tensor("xt_cl", [3, GPC * NC], F32, kind="ExternalInput")
    d_liw = nc.dram_tensor("li_w3", [3, H], F32, kind="ExternalInput")
    d_ciw = nc.dram_tensor("ci_w3", [3, H], F32, kind="ExternalInput")
    d_out = nc.dram_tensor("vote", [1, GPC * NL], F32, kind="ExternalOutput")
    if DEBUG_DUMP:
        d_dbg_lh = nc.dram_tensor("dbg_lh", [ITERS, GPC, LB, 512], F32,
                                  kind="ExternalOutput")
        d_dbg_ch = nc.dram_tensor("dbg_ch", [ITERS, GPC, CB, 256], F32,
                                  kind="ExternalOutput")
        d_dbg_xl = nc.dram_tensor("dbg_xl", [ITERS, GPC, 128, NL], F32,
                                  kind="ExternalOutput")
        d_dbg_xc = nc.dram_tensor("dbg_xc", [ITERS, GPC, 128, NC], F32,
                                  kind="ExternalOutput")

    with TileContext(nc) as tc, ExitStack() as ctx:
        const = ctx.enter_context(tc.tile_pool(name="const", bufs=1))
        state = ctx.enter_context(tc.tile_pool(name="state", bufs=2))
        work = ctx.enter_context(tc.tile_pool(name="work", bufs=2))
        ps = ctx.enter_context(tc.tile_pool(name="ps", bufs=1, space="PSUM"))

        ident = const.tile([128, 128], F32, name="ident")
        make_identity(nc, ident)

        def load(dram, shape, nm, dt=F32):
            t = const.tile(shape, dt, name=nm)
            nc.sync.dma_start(out=t[:, :], in_=dram[:, :])
            return t

        # init-critical first: xt/init weights, then fp32-phase consts;
        # tail-only bf16/f32r duplicates stream last (needed at t>=TR only)
        xt_lit = load(d_xt_lit, [3, GPC * NL], "xt_lit_sb")
        xt_cl = load(d_xt_cl, [3, GPC * NC], "xt_cl_sb")
        li_w3 = load(d_liw, [3, H], "li_w3_sb")
        ci_w3 = load(d_ciw, [3, H], "ci_w3_sb")
        bias_cl = load(d_bias_cl, [CB, 1024], "bias_cl_sb")
        bias_lit = const.tile([LB, GPC * 2048], F32, name="bias_lit_sb")
        for g in range(GPC):
            nc.sync.dma_start(
                out=bias_lit[:, g * 2048:(g + 1) * 2048], in_=d_bias_lit[g])
        wc1 = load(d_wc1, [128, 256], "wc1_sb")
        wc = load(d_wc, [128, 256], "wc_sb")
        wl1 = load(d_wl1, [128, 256], "wl1_sb")
        wl = load(d_wl, [128, 256], "wl_sb")
        wf = load(d_wf, [128, 256], "wf_sb")
        atc = const.tile([LB, GPC * NLB * NC], F32, name="atc_sb")
        for g in range(GPC):
            for k in range(NLB):
                c0 = (g * NLB + k) * NC
                nc.sync.dma_start(out=atc[:, c0:c0 + NC], in_=d_atc[g, k])
        alc = const.tile([CB, GPC * NCB * NL], F32, name="alc_sb")
        for g in range(GPC):
            for j in range(NCB):
                c0 = (g * NCB + j) * NL
                nc.sync.dma_start(out=alc[:, c0:c0 + NL], in_=d_alc[g, j])
        atc_b = const.tile([LB, GPC * NLB * NC], BF16, name="atc_b_sb")
        for g in range(GPC):
            for k in range(NLB):
                c0 = (g * NLB + k) * NC
                nc.sync.dma_start(out=atc_b[:, c0:c0 + NC], in_=d_atc_b[g, k])
        alc_b = const.tile([CB, GPC * NCB * NL], BF16, name="alc_b_sb")
        for g in range(GPC):
            for j in range(NCB):
                c0 = (g * NCB + j) * NL
                nc.sync.dma_start(out=alc_b[:, c0:c0 + NL], in_=d_alc_b[g, j])
        wc_r = load(d_wc_r, [128, 256], "wc_r_sb", F32R)
        wl_r = load(d_wl_r, [128, 256], "wl_r_sb", F32R)
        wf_r = load(d_wf_r, [128, 256], "wf_r_sb", F32R)
        wv_r = load(d_wv_r, [128, 1], "wv_r_sb", F32R)
        wv_f = const.tile([128, 1], F32, name="wv_f_sb")
        nc.vector.tensor_copy(wv_f[:, :], wv_r[:, :])
        bv = load(d_bv, [1, 1], "bv_sb")

        def MM(*a, **kw):
            kw.setdefault("skip_group_check", True)
            return nc.tensor.matmul(*a, **kw)

        # engine rotation for psum-touching copies: ACT/DVE only
        # (GPSIMD instructions cannot access PSUM on trn2)
        _cyc = [0]
        COPY_ENGINES = (nc.scalar.copy, nc.vector.tensor_copy)

        def rcopy(out, in_):
            f = COPY_ENGINES[_cyc[0] % len(COPY_ENGINES)]
            _cyc[0] += 1
            f(out, in_)

        # preloads may only come from ACT or DVE (gpsimd->psum is broken)
        _pcyc = [0]

        def preload(out, in_):
            _pcyc[0] += 1
            return nc.scalar.copy(out, in_)

        def b3(ap, b):
            return ap.rearrange("p (b z) -> p b z", b=b)

        # ---------- state transposes: h_rm -> fm halves of next x tiles ----
        def make_x(t, g, lh_rm, ch_rm, with_cl=True):
            """Allocate xl/xc for iteration t and fill rows 64:128 with the
            transposed h state. Rows 0:64 are filled by the agg phases."""
            dt = F32R if t >= TR_GATE else F32
            xc = None
            if with_cl:
                xc = state.tile([128, NC], dt, tag=f"xc{g}", name=f"xc{g}_{t}")
                ptc = ps.tile([64, NC], F32, tag="pt", bufs=2,
                              name=f"ptc_{t}_{g}")
                for mb in range(NCB):
                    nc.tensor.transpose(
                        ptc[:, mb * CB:(mb + 1) * CB],
                        ch_rm[:, mb * H:(mb + 1) * H],
                        ident[0:CB, 0:CB],
                    )
                rcopy(xc[64:128, :], ptc[:, :])
            xl = state.tile([128, NL], dt, tag=f"xl{g}", name=f"xl{g}_{t}")
            ptr0 = ps.tile([64, 400], F32, tag="pt", bufs=2, name=f"ptl0_{t}_{g}")
            ptr1 = ps.tile([64, 400], F32, tag="pt", bufs=2, name=f"ptl1_{t}_{g}")
            for mb in range(NLB):
                dst = (ptr0, ptr1)[mb // 4]
                nc.tensor.transpose(
                    dst[:, (mb % 4) * LB:(mb % 4 + 1) * LB],
                    lh_rm[:, mb * H:(mb + 1) * H],
                    ident[0:LB, 0:LB],
                )
            rcopy(xl[64:128, 0:400], ptr0[:, :])
            rcopy(xl[64:128, 400:800], ptr1[:, :])
            return xl, xc

        # warm up both gate-psum ring slots with a start/stop group: the
        # accumulate-onto-engine-preload pattern is unreliable on a psum
        # bank's first-ever touch (has_written state undefined until a
        # matmul group has start/stopped there).
        for slot in range(2):
            pgw = ps.tile([128, 1024], F32, tag="pg", bufs=2,
                          name=f"pgwarm_{slot}")
            for h in range(2):
                MM(pgw[:, h * 512:(h + 1) * 512], ident[0:1, 0:128],
                   bias_cl[0:1, h * 512:(h + 1) * 512],
                   start=True, stop=True)

        # ---------- initial states (rm) ----------
        lh_cur, ch_cur = {}, {}
        lh_b_cur, ch_b_cur = {}, {}
        lc_cur, cc_cur = {g: None for g in range(GPC)}, {g: None for g in range(GPC)}
        for g in range(GPC):
            pl = ps.tile([LB, 512], F32, tag="pa", bufs=2, name=f"inl_{g}")
            for mb in range(NLB):
                MM(pl[:, mb * H:(mb + 1) * H],
                   xt_lit[0:3, g * NL + mb * LB:g * NL + (mb + 1) * LB],
                   li_w3[0:3, :], start=True, stop=True)
            lh0 = state.tile([LB, 512], F32, tag=f"lh{g}", name=f"lh{g}_0")
            nc.scalar.copy(lh0[:, :], pl[:, :])
            pc = ps.tile([CB, 256], F32, tag="pa", bufs=2, name=f"inc_{g}")
            for mb in range(NCB):
                MM(pc[:, mb * H:(mb + 1) * H],
                   xt_cl[0:3, g * NC + mb * CB:g * NC + (mb + 1) * CB],
                   ci_w3[0:3, :], start=True, stop=True)
            ch0 = state.tile([CB, 256], F32, tag=f"ch{g}", name=f"ch{g}_0")
            nc.scalar.copy(ch0[:, :], pc[:, :])
            lh_cur[g], ch_cur[g] = lh0, ch0

        xl_cur, xc_cur = {}, {}
        for g in range(GPC):
            xl_cur[g], xc_cur[g] = make_x(1, g, lh_cur[g], ch_cur[g])

        # ---------- iterations ----------
        def emit_phase_a(t, g):
            """Clause agg of iteration t, graph g -> xc_cur[g][0:64]."""
            if t < TR_AGG:
                pa = ps.tile([CB, 256], F32, tag="pa", bufs=2,
                             name=f"pac_{t}_{g}")
                for mb in range(NCB):
                    for k in range(NLB):
                        c0 = (g * NLB + k) * NC + mb * CB
                        MM(pa[:, mb * H:(mb + 1) * H],
                           atc[:, c0:c0 + CB],
                           lh_cur[g][:, k * H:(k + 1) * H],
                           start=(k == 0), stop=(k == NLB - 1))
                a_sb = work.tile([CB, 256], F32, tag="agsb",
                                 name=f"asb_{t}_{g}")
                rcopy(a_sb[:, :], pa[:, :])
                pat = ps.tile([64, NC], F32, tag="pt", bufs=2,
                              name=f"pat_{t}_{g}")
                for mb in range(NCB):
                    nc.tensor.transpose(
                        pat[:, mb * CB:(mb + 1) * CB],
                        a_sb[:, mb * H:(mb + 1) * H],
                        ident[0:CB, 0:CB],
                    )
                rcopy(xc_cur[g][0:64, :], pat[:, :])
            else:
                pfa = ps.tile([64, NC], F32, tag="pt", bufs=2,
                              name=f"pfa_{t}_{g}")
                for k in range(NLB):
                    c0 = (g * NLB + k) * NC
                    MM(pfa[:, :],
                       lh_b_cur[g][:, k * H:(k + 1) * H],
                       atc_b[:, c0:c0 + NC],
                       start=(k == 0), stop=(k == NLB - 1))
                rcopy(xc_cur[g][0:64, :], pfa[:, :])

        emit_phase_a(1, 0)

        def emit_phase_b(t, g):
            """Clause gates + LSTM of iteration t, graph g."""
            tail_a = t >= TR_AGG
            tail_g = t >= TR_GATE
            if True:
                pgc = ps.tile([CB, 1024], F32, tag="pg", bufs=2,
                              name=f"pgc_{t}_{g}")
                pre = preload(pgc[:, :], bias_cl[:, :])
                W = wc1 if t == 1 else (wc_r if tail_g else wc)
                for mb in range(NCB):
                    mm = MM(pgc[:, mb * 256:(mb + 1) * 256],
                            xc_cur[g][:, mb * CB:(mb + 1) * CB],
                            W[:, :], start=False, stop=True)
                    if mb == 0:
                        add_dep_helper(mm.ins, pre.ins, sync=True,
                                       reason="gates accumulate onto bias")
                S = work.tile([CB, 1024], F32, tag=f"Sc{g}", bufs=1,
                              name=f"Sc_{t}_{g}")
                nc.scalar.activation(S[:, :], pgc[:, :], SIG)
                t1 = work.tile([CB, 256], F32, tag="t1", name=f"ct1_{t}_{g}")
                cc_new = state.tile([CB, 256], F32, tag=f"cc{g}",
                                    name=f"cc{g}_{t}")
                ch_new = state.tile([CB, 256], F32, tag=f"ch{g}",
                                    name=f"ch{g}_{t}")
                for hf in range(2):
                    hs = slice(hf * 2, (hf + 1) * 2)
                    si = b3(S[:, :], 4)[:, hs, 0:64]
                    sf = b3(S[:, :], 4)[:, hs, 64:128]
                    sg = b3(S[:, :], 4)[:, hs, 128:192]
                    so = b3(S[:, :], 4)[:, hs, 192:256]
                    t1v = b3(t1[:, :], 4)[:, hs, :]
                    nc.vector.tensor_mul(t1v, si, sg)
                    ccv = b3(cc_new[:, :], 4)[:, hs, :]
                    if t == 1:
                        nc.vector.scalar_tensor_tensor(
                            ccv, t1v, 2.0, si, op0=MULT, op1=SUB)
                    else:
                        nc.vector.scalar_tensor_tensor(
                            t1v, t1v, 2.0, si, op0=MULT, op1=SUB)
                        nc.vector.tensor_mul(
                            sf, sf, b3(cc_cur[g][:, :], 4)[:, hs, :])
                        nc.vector.tensor_add(ccv, t1v, sf)
                    nc.scalar.activation(sg, ccv, SIG, scale=2.0)
                    nc.vector.tensor_mul(t1v, so, sg)
                    nc.vector.scalar_tensor_tensor(
                        b3(ch_new[:, :], 4)[:, hs, :], t1v, 2.0, so,
                        op0=MULT, op1=SUB)
                cc_cur[g], ch_cur[g] = cc_new, ch_new
                if tail_a:
                    chb = state.tile([CB, 256], BF16, tag=f"chb{g}", bufs=1,
                                     name=f"chb{g}_{t}")
                    nc.gpsimd.tensor_copy(chb[:, :], ch_new[:, :])
                    ch_b_cur[g] = chb

        emit_phase_a(1, 0)
        emit_phase_b(1, 0)

        for t in range(1, ITERS):
            tail_a = t >= TR_AGG
            tail_g = t >= TR_GATE

            emit_phase_a(t, 1)
            emit_phase_b(t, 1)

            # -- phase C: lit agg -> xl[0:64] --
            for g in range(GPC):
                if not tail_a:
                    pa = ps.tile([LB, 512], F32, tag="pa", bufs=2,
                                 name=f"pal_{t}_{g}")
                    for mb in range(NLB):
                        for j in range(NCB):
                            c0 = (g * NCB + j) * NL + mb * LB
                            MM(pa[:, mb * H:(mb + 1) * H],
                               alc[:, c0:c0 + LB],
                               ch_cur[g][:, j * H:(j + 1) * H],
                               start=(j == 0), stop=(j == NCB - 1))
                    c_sb = work.tile([LB, 512], F32, tag="agsb",
                                     name=f"csb_{t}_{g}")
                    rcopy(c_sb[:, :], pa[:, :])
                    ptl0 = ps.tile([64, 400], F32, tag="pt", bufs=2,
                                   name=f"pbl0_{t}_{g}")
                    ptl1 = ps.tile([64, 400], F32, tag="pt", bufs=2,
                                   name=f"pbl1_{t}_{g}")
                    for mb in range(NLB):
                        dst = (ptl0, ptl1)[mb // 4]
                        nc.tensor.transpose(
                            dst[:, (mb % 4) * LB:(mb % 4 + 1) * LB],
                            c_sb[:, mb * H:(mb + 1) * H],
                            ident[0:LB, 0:LB],
                        )
                    rcopy(xl_cur[g][0:64, 0:400], ptl0[:, :])
                    rcopy(xl_cur[g][0:64, 400:800], ptl1[:, :])
                else:
                    for hf in range(2):
                        pfl = ps.tile([64, 400], F32, tag="pt", bufs=2,
                                      name=f"pfl_{t}_{g}_{hf}")
                        for j in range(NCB):
                            c0 = (g * NCB + j) * NL + hf * 400
                            MM(pfl[:, :],
                               ch_b_cur[g][:, j * H:(j + 1) * H],
                               alc_b[:, c0:c0 + 400],
                               start=(j == 0), stop=(j == NCB - 1))
                        rcopy(xl_cur[g][0:64, hf * 400:(hf + 1) * 400],
                              pfl[:, :])

            # -- phase D: lit gates + LSTM --
            for g in range(GPC):
                S = work.tile([LB, 2048], F32, tag=f"Sl{g}", bufs=1,
                              name=f"Sl_{t}_{g}")
                WL = wl1 if t == 1 else (wl_r if tail_g else wl)
                WF = wf_r if tail_g else wf
                for hf in range(2):
                    pgl = ps.tile([LB, 1024], F32, tag="pg", bufs=2,
                                  name=f"pgl_{t}_{g}_{hf}")
                    pre = preload(pgl[:, :],
                                  bias_lit[:, g * 2048 + hf * 1024:
                                           g * 2048 + (hf + 1) * 1024])
                    for mbl in range(4):
                        mb = hf * 4 + mbl
                        mm = MM(pgl[:, mbl * 256:(mbl + 1) * 256],
                                xl_cur[g][:, mb * LB:(mb + 1) * LB],
                                WL[:, :], start=False, stop=False)
                        if mbl == 0:
                            add_dep_helper(mm.ins, pre.ins, sync=True,
                                           reason="gates accumulate onto bias")
                        fb = ((mb + 4) % NLB) * LB
                        MM(pgl[:, mbl * 256:(mbl + 1) * 256],
                           xl_cur[g][64:128, fb:fb + LB],
                           WF[64:128, :], start=False, stop=True)
                    nc.scalar.activation(
                        S[:, hf * 1024:(hf + 1) * 1024], pgl[:, :], SIG)
                t1 = work.tile([LB, 512], F32, tag="t1", name=f"lt1_{t}_{g}")
                lc_new = state.tile([LB, 512], F32, tag=f"lc{g}",
                                    name=f"lc{g}_{t}")
                lh_new = state.tile([LB, 512], F32, tag=f"lh{g}",
                                    name=f"lh{g}_{t}")
                for hf in range(2):
                    hs = slice(hf * 4, (hf + 1) * 4)
                    si = b3(S[:, :], 8)[:, hs, 0:64]
                    sf = b3(S[:, :], 8)[:, hs, 64:128]
                    sg = b3(S[:, :], 8)[:, hs, 128:192]
                    so = b3(S[:, :], 8)[:, hs, 192:256]
                    t1v = b3(t1[:, :], 8)[:, hs, :]
                    nc.vector.tensor_mul(t1v, si, sg)
                    lcv = b3(lc_new[:, :], 8)[:, hs, :]
                    if t == 1:
                        nc.vector.scalar_tensor_tensor(
                            lcv, t1v, 2.0, si, op0=MULT, op1=SUB)
                    else:
                        nc.vector.scalar_tensor_tensor(
                            t1v, t1v, 2.0, si, op0=MULT, op1=SUB)
                        nc.vector.tensor_mul(
                            sf, sf, b3(lc_cur[g][:, :], 8)[:, hs, :])
                        nc.vector.tensor_add(lcv, t1v, sf)
                    nc.scalar.activation(sg, lcv, SIG, scale=2.0)
                    nc.vector.tensor_mul(t1v, so, sg)
                    nc.vector.scalar_tensor_tensor(
                        b3(lh_new[:, :], 8)[:, hs, :], t1v, 2.0, so,
                        op0=MULT, op1=SUB)
                lc_cur[g], lh_cur[g] = lc_new, lh_new
                if t + 1 >= TR_AGG:
                    lhb = state.tile([LB, 512], BF16, tag=f"lhb{g}", bufs=1,
                                     name=f"lhb{g}_{t}")
                    nc.gpsimd.tensor_copy(lhb[:, :], lh_new[:, :])
                    lh_b_cur[g] = lhb

            if DEBUG_DUMP:
                for g in range(GPC):
                    nc.sync.dma_start(out=d_dbg_lh[t, g], in_=lh_cur[g][:, :])
                    nc.sync.dma_start(out=d_dbg_ch[t, g], in_=ch_cur[g][:, :])
                    xlw = work.tile([128, NL], F32, tag="dbgx", name=f"dbgxl_{t}_{g}")
                    nc.vector.tensor_copy(xlw[:, :], xl_cur[g][:, :])
                    nc.sync.dma_start(out=d_dbg_xl[t, g], in_=xlw[:, :])
                    xcw = work.tile([128, NC], F32, tag="dbgxc", name=f"dbgxc_{t}_{g}")
                    nc.vector.tensor_copy(xcw[:, :], xc_cur[g][:, :])
                    nc.sync.dma_start(out=d_dbg_xc[t, g], in_=xcw[:, :])

            # -- phase E: transposes for t+1, pipelined with A/B(t+1, g0) --
            if t < ITERS - 1:
                xl_cur[0], xc_cur[0] = make_x(t + 1, 0, lh_cur[0], ch_cur[0])
                emit_phase_a(t + 1, 0)
                emit_phase_b(t + 1, 0)
                xl_cur[1], xc_cur[1] = make_x(t + 1, 1, lh_cur[1], ch_cur[1])
            else:
                for g in range(GPC):
                    xl_cur[g], xc_cur[g] = make_x(
                        t + 1, g, lh_cur[g], ch_cur[g], with_cl=False)

        # ---- vote head: xl_24[64:128] holds lh fm (F32R) ----
        wv_use = wv_r if ITERS >= TR_GATE else wv_f
        for g in range(GPC):
            for hf in range(2):
                pv = ps.tile([1, 400], F32, tag="pa", bufs=2,
                             name=f"pv_{g}_{hf}")
                MM(pv[:, :], wv_use[64:128, 0:1],
                   xl_cur[g][64:128, hf * 400:(hf + 1) * 400],
                   start=True, stop=True)
                vo = work.tile([1, 400], F32, tag="vote", name=f"vo_{g}_{hf}")
                nc.scalar.activation(vo[0:1, :], pv[:, :], IDENT,
                                     bias=bv[0:1, 0:1])
                nc.sync.dma_start(
                    out=d_out[0:1, g * NL + hf * 400:g * NL + (hf + 1) * 400],
                    in_=vo[0:1, :])

    nc.compile()
    return nc


def _fold_and_shard(inputs):
    """Host-side preprocessing: fold weights, build adjacency, shard by graph."""
    f32 = np.float32
    import ml_dtypes
    bf16 = ml_dtypes.bfloat16
    g = {k: np.asarray(v) for k, v in inputs.items()}

    def collapse(w1, b1, w2, b2, w3, b3):
        return w1 @ w2 @ w3, ((b1 @ w2) + b2) @ w3 + b3

    Wl, bl = collapse(g["lm1_w"], g["lm1_b"], g["lm2_w"], g["lm2_b"],
                      g["lm3_w"], g["lm3_b"])
    Wc, bc = collapse(g["cm1_w"], g["cm1_b"], g["cm2_w"], g["cm2_b"],
                      g["cm3_w"], g["cm3_b"])
    Wv, bv = collapse(g["lv1_w"], g["lv1_b"], g["lv2_w"], g["lv2_b"],
                      g["lv3_w"], g["lv3_b"])

    cu_wih, lu_wih = g["cu_wih"], g["lu_wih"]
    w_lc = (Wl @ cu_wih).astype(f32)                 # agg_c -> clause gates
    w_ch = (w_lc + g["cu_whh"]).astype(f32)          # t>=2 merged recurrent
    cbias_c = ((K + 1) * (bl @ cu_wih) + g["cu_bih"] + g["cu_bhh"]).astype(f32)
    wih_a = lu_wih[0:H].astype(f32)                  # flip -> lit gates
    w_cl2 = (Wc @ lu_wih[H:2 * H]).astype(f32)       # agg_l -> lit gates
    w_lh = (w_cl2 + g["lu_whh"]).astype(f32)         # t>=2 merged recurrent
    q_l = (bc @ lu_wih[H:2 * H]).astype(f32)         # [256]
    cbias_l = (g["lu_bih"] + g["lu_bhh"]).astype(f32)

    def gscale(w):
        """double the g-gate columns (128:192) so one sigmoid serves all
        four gates (tanh(x) = 2*sig(2x)-1)."""
        w = w.copy()
        w[..., 128:192] *= 2.0
        return w

    vs = np.vstack
    wc1 = gscale(vs([w_lc, w_lc]))
    wc_m = gscale(vs([w_lc, w_ch]))
    wl1 = gscale(vs([w_cl2, w_cl2]))
    wl_m = gscale(vs([w_cl2, w_lh]))
    wf_m = gscale(wih_a)
    cbias_c_s = gscale(cbias_c)
    bias_cl = np.tile(cbias_c_s[None, :], (CB, 4)).astype(f32)  # [110, 1024]

    li_w3 = np.concatenate([g["li_w"], g["li_b"][None, :]], axis=0).astype(f32)
    ci_w3 = np.concatenate([g["ci_w"], g["ci_b"][None, :]], axis=0).astype(f32)

    # adjacency per graph from edge_index (direction-robust)
    ei = g["edge_index"].astype(np.int64)
    src, dst = ei[0], ei[1]
    src_g, dst_g = src // NPG, dst // NPG
    assert np.all(src_g == dst_g), "edges must be graph-local"
    src_l, dst_l = src % NPG, dst % NPG
    s_lit, d_lit = src_l < NL, dst_l < NL
    A_in_c = np.zeros((B, NC, NL), f32)   # clause <- literal edges
    m = (~d_lit) & s_lit
    np.add.at(A_in_c, (dst_g[m], dst_l[m] - NL, src_l[m]), 1.0)
    A_in_l = np.zeros((B, NL, NC), f32)   # literal <- clause edges
    m = d_lit & (~s_lit)
    np.add.at(A_in_l, (dst_g[m], dst_l[m], src_l[m] - NL), 1.0)
    deg_l = A_in_l.sum(axis=2)            # [B, NL]

    x = g["x"].astype(f32).reshape(B, NPG, 2)
    ones = np.ones((B, NPG, 1), f32)
    x3 = np.concatenate([x, ones], axis=2)        # [B, NPG, 3]

    q_s = gscale(q_l)
    cb_s = gscale(cbias_l)

    wf128 = np.vstack([wf_m, wf_m]).astype(f32)          # rows 64:128 used
    wv128 = np.vstack([Wv, Wv]).astype(f32)
    shared = dict(
        wc1=wc1, wc=wc_m, wc_r=wc_m, wl1=wl1, wl=wl_m, wl_r=wl_m,
        wf=wf128, wf_r=wf128, wv_r=wv128,
        bias_cl=bias_cl, bv=np.array([[bv[0]]], f32),
        li_w3=li_w3, ci_w3=ci_w3,
    )
    in_maps = []
    for c in range(NCORES):
        atc = np.zeros((GPC, NLB, LB, NC), f32)
        alc = np.zeros((GPC, NCB, CB, NL), f32)
        bias_lit = np.zeros((GPC, LB, 2048), f32)
        for gg in range(GPC):
            gid = c * GPC + gg
            at_full = A_in_c[gid].T              # [NL, NC]
            al_full = A_in_l[gid].T              # [NC, NL]
            for k in range(NLB):
                atc[gg, k] = at_full[LB * k:LB * (k + 1)]
            for j in range(NCB):
                alc[gg, j] = al_full[CB * j:CB * (j + 1)]
            # bias_lit[g][p, mb*256 + j] = q_s[j]*(deg[mb*100+p]+1) + cb_s[j]
            d1 = deg_l[gid] + 1.0                # [800]
            bl_g = (d1[:, None] * q_s[None, :] + cb_s[None, :])  # [800, 256]
            bias_lit[gg] = bl_g.reshape(NLB, LB, 256).transpose(1, 0, 2).reshape(LB, 2048)
        gs = slice(c * GPC, (c + 1) * GPC)
        x3c = x3[gs]                               # [GPC, NPG, 3]
        xt_lit = np.ascontiguousarray(
            x3c[:, :NL].transpose(2, 0, 1).reshape(3, GPC * NL))
        xt_cl = np.ascontiguousarray(
            x3c[:, NL:].transpose(2, 0, 1).reshape(3, GPC * NC))
        in_maps.append(dict(
            atc=atc, atc_b=atc.astype(bf16), alc=alc, alc_b=alc.astype(bf16),
            bias_lit=bias_lit, xt_lit=xt_lit, xt_cl=xt_cl, **shared,
        ))
    return in_maps


_LAST_RESULTS = {}


def kernel(**inputs):
    from concourse.bass_utils import run_bass_kernel_spmd

    in_maps = _fold_and_shard(inputs)
    if "nc" not in _PROGRAM_CACHE:
        _PROGRAM_CACHE["nc"] = _build_program()
    nc = _PROGRAM_CACHE["nc"]
    res = run_bass_kernel_spmd(nc, in_maps, core_ids=list(range(NCORES)))
    _LAST_RESULTS["res"] = res
    out = np.zeros((N, 1), np.float32)
    for c in range(NCORES):
        vote = res.results[c]["vote"].reshape(GPC, NL)
        for g in range(GPC):
            base = (c * GPC + g) * NPG
            out[base:base + NL, 0] = vote[g]
    return out
